# revision 1
# baseline (speedup 1.0000x reference)
"""Trainium2 Bass kernel for DPLossV2 soft-rank MSE loss.

Computes, for x:[512,512], z:[512,64]:
    dist_x = cdist(x), dist_z = cdist(z)           (pairwise Euclidean)
    rank_m[i,j] = 1 + sum_k sigmoid((m[i,k]-m[i,j])/tau)
    loss = mean((rank_z - rank_x)**2)
returns (loss, loss, 0.0) since lambda_rank=1, lambda_pairdist=0.

Sharding: the 512 rows of both distance matrices split across 8
NeuronCores (64 rows each). Per core, the x-row slab occupies SBUF
partitions 0-63 and the z-row slab partitions 64-127, so every ScalarE
instruction processes both matrices at once (full 128-lane utilization).

The O(n^3) soft-rank is done triangularly: instruction k evaluates
    T_k[p, j] = sigmoid(S[p,k] - S[p,j])    for j < k only
(ACT with per-partition bias S[:,k], scale=-1, PSUM-sourced input),
which halves the sigmoid payload; the j > k half follows from
sigmoid(u) + sigmoid(-u) = 1:
    rank[p,j] = 1.5 + j + sum_{k>j} T_k[p,j] - sum_{k<j} T_j[p,k]
PE accumulates the first (cross-instruction) sum into PSUM via
identity-matmuls; VectorE row-reduces each T_k into W[:,k] for the
second. The device outputs V = R_acc - W per core; the host forms
D = V[z-half] - V[x-half] (the 1.5+j terms cancel) and the scalar MSE
partial sums are reduced across the 8 cores in float64.

Hardware-measured: ~287 us on trn2 (ScalarE-bound: 511 sigmoid ACTs at
~300ns fixed + ~1ns/elem; this is the structural floor for the
per-k-bias formulation since a core's 128 partitions x 512 instruction
slots exactly cover its 1024 row-matrix units x 512 columns).
"""

import numpy as np
from contextlib import ExitStack

import concourse.bass as bass
import concourse.bacc as bacc
import concourse.mybir as mybir
import concourse.tile as tile
from concourse.bass_utils import run_bass_kernel_spmd

N = 512        # number of rows / rank dimension
DX = 512       # x feature dim
DZ = 64        # z feature dim
NCORES = 8
ROWS = N // NCORES          # 64 rows per core
F32 = mybir.dt.float32
BF16 = mybir.dt.bfloat16
AF = mybir.ActivationFunctionType
TAU = 1.0


def _build() -> bass.Bass:
    nc = bacc.Bacc()

    # Per-core inputs. Each matmul must depend on a single DMA (the LDW
    # sync-wait slot is limited), so rhs|lhsT are concatenated per tensor:
    # columns 0..N-1 = full transposed matrix (rhs), N..N+ROWS-1 = this
    # core's slab columns (lhsT). The two aux contraction rows fold the
    # squared norms into the matmul: G' = x_i.x_j - sq_i/2 - sq_j/2.
    W = N + ROWS
    xcat = nc.dram_tensor("xcat", [DX, W], F32, kind="ExternalInput")
    zcat = nc.dram_tensor("zcat", [DZ, W], F32, kind="ExternalInput")
    acx = nc.dram_tensor("acx", [2, W], F32, kind="ExternalInput")
    acz = nc.dram_tensor("acz", [2, W], F32, kind="ExternalInput")
    ident = nc.dram_tensor("ident", [128, 128], F32, kind="ExternalInput")
    rout = nc.dram_tensor("rout", [128, N], F32, kind="ExternalOutput")

    nb = DX // 128  # xcat partition blocks

    with tile.TileContext(nc) as tc:
        with ExitStack() as ctx:
            cp = ctx.enter_context(tc.tile_pool(name="const", bufs=1))
            tkp = ctx.enter_context(tc.tile_pool(name="tk", bufs=16))
            pp = ctx.enter_context(tc.tile_pool(name="ps", bufs=1, space="PSUM"))

            xb = [cp.tile([128, W], F32, name=f"xb{b}", tag=f"xb{b}") for b in range(nb)]
            zb = cp.tile([DZ, W], F32, tag="zb")
            ax = cp.tile([2, W], F32, tag="ax")
            az = cp.tile([2, W], F32, tag="az")
            idf = cp.tile([128, 128], F32, tag="idf")
            id_sb = cp.tile([128, 128], BF16, tag="ident")
            s_sb = cp.tile([128, N], F32, tag="s_sb")    # stacked distances
            rr_sb = cp.tile([128, N], F32, tag="rr")

            for b in range(nb):
                nc.sync.dma_start(xb[b][0:64, :], xcat[b * 128:b * 128 + 64, :])
                nc.sync.dma_start(xb[b][64:128, :], xcat[b * 128 + 64:(b + 1) * 128, :])
            nc.sync.dma_start(zb[0:32, :], zcat[0:32, :])
            nc.sync.dma_start(zb[32:DZ, :], zcat[32:DZ, :])
            nc.sync.dma_start(ax[:], acx[:])
            nc.sync.dma_start(az[:], acz[:])
            nc.sync.dma_start(idf[:], ident[:])
            # identity -> bf16 via ScalarE so the k-loop matmuls depend only
            # on the ScalarE semaphore (one wait per matmul)
            nc.scalar.copy(id_sb[:], idf[:])

            g_s = pp.tile([128, N], F32, tag="g_s")
            s_ps = pp.tile([128, N], F32, tag="s_ps")
            r_ps = pp.tile([128, N], F32, tag="r_ps")

            # G' matmuls (contraction over features + 2 aux rows).
            # x-rows land on PSUM partitions 0-63, z-rows on 64-127 via
            # PE column tiling, so one ACT covers both distance slabs.
            for b in range(nb):
                nc.tensor.matmul(g_s[0:ROWS, :], xb[b][:, N:W], xb[b][:, 0:N],
                                 start=(b == 0), stop=False)
            nc.tensor.matmul(g_s[0:ROWS, :], ax[:, N:W], ax[:, 0:N],
                             start=False, stop=True)
            nc.tensor.matmul(g_s[ROWS:2 * ROWS, :], zb[:, N:W], zb[:, 0:N],
                             start=True, stop=False, tile_position=(0, ROWS))
            nc.tensor.matmul(g_s[ROWS:2 * ROWS, :], az[:, N:W], az[:, 0:N],
                             start=False, stop=True, tile_position=(0, ROWS))

            # distances: S = sqrt(max(-2 G', 0)) for both stacked slabs
            # (clamp on VectorE: keeps ScalarE to two ACT table sets)
            nc.vector.tensor_scalar(rr_sb[:], g_s[:], -2.0 / (TAU * TAU), 0.0,
                                    mybir.AluOpType.mult, mybir.AluOpType.max)
            nc.scalar.activation(s_sb[:], rr_sb[:], AF.Sqrt)
            # PSUM copy of S: ACT reads are cheaper from PSUM (172 vs 222 cyc)
            nc.vector.tensor_copy(s_ps[:], s_sb[:])

            # Dummy 1-elem sigmoid: forces the sigmoid ACT-table load here,
            # so the first real sigmoid doesn't pay an implicit table-load
            # (which costs it a sync-wait slot in walrus codegen).
            warm = cp.tile([1, 1], F32, tag="warm")
            nc.scalar.activation(warm[:], rr_sb[0:1, 0:1], AF.Sigmoid)

            # O(n^3) soft-rank, triangular: instruction k computes
            #   T_k[p, j] = sigmoid(S[p,k] - S[p,j])   for j < k only.
            # The j > k half follows from sigmoid(u) + sigmoid(-u) = 1:
            #   rank[p,j] = 1.5 + j + sum_{k>j} T_k[p,j] - sum_{k<j} T_j[p,k]
            # PE accumulates the first (cross-instruction) sum into r_ps;
            # VectorE row-reduces each T_k into w_sb[:, k] for the second.
            # The (1.5 + j) terms cancel in rank_z - rank_x on the host.
            w_sb = cp.tile([128, N], F32, tag="w_sb")
            nc.vector.memset(w_sb[:, 0:1], 0.0)
            # k descends so the first (start=True) matmul covers the widest
            # PSUM region; later ones write subsets of already-initialized
            # columns (PSUM lazy-zeroing is per start-region).
            for k in range(N - 1, 0, -1):
                tk = tkp.tile([128, k], BF16, name="tk", tag="tk")
                nc.scalar.activation(tk[:], s_ps[:, 0:k], AF.Sigmoid,
                                     bias=s_sb[:, k:k + 1], scale=-1.0)
                nc.vector.tensor_reduce(w_sb[:, k:k + 1], tk[:],
                                        axis=mybir.AxisListType.X,
                                        op=mybir.AluOpType.add)
                nc.tensor.matmul(r_ps[:, 0:k], id_sb[:], tk[:],
                                 start=(k == N - 1), stop=(k == 1))

            # V = R_acc - W ; host computes D = V[64:] - V[:64]
            r_sb = cp.tile([128, N], F32, tag="r_sb")
            # column N-1 has no k>j terms: V = 0 - W there (avoids touching
            # the PSUM accumulator bank from another engine mid-group)
            nc.vector.tensor_sub(r_sb[:, 0:N - 1], r_ps[:, 0:N - 1],
                                 w_sb[:, 0:N - 1])
            nc.vector.tensor_scalar_mul(r_sb[:, N - 1:N], w_sb[:, N - 1:N], -1.0)
            # SWDGE: static HWDGE DMAs have a single sync-wait slot, and this
            # one needs waits on both the DVE copy and the DMA queue clock.
            nc.gpsimd.dma_start(rout[:], r_sb[:])

    nc.compile()
    return nc


_CACHE: dict = {}


def _get_nc() -> bass.Bass:
    if "nc" not in _CACHE:
        _CACHE["nc"] = _build()
    return _CACHE["nc"]


def make_in_maps(x: np.ndarray, z: np.ndarray) -> list[dict]:
    x = np.ascontiguousarray(np.asarray(x, np.float32))
    z = np.ascontiguousarray(np.asarray(z, np.float32))
    sqx = (x * x).sum(1, dtype=np.float32)
    sqz = (z * z).sum(1, dtype=np.float32)
    xt = np.ascontiguousarray(x.T)
    zt = np.ascontiguousarray(z.T)
    axr = np.stack([np.ones(N, np.float32), sqx])
    azr = np.stack([np.ones(N, np.float32), sqz])
    ident = np.eye(128, dtype=np.float32)
    in_maps = []
    for c in range(NCORES):
        s = slice(c * ROWS, (c + 1) * ROWS)
        axl = np.stack([-sqx[s] / 2, np.full(ROWS, -0.5, np.float32)])
        azl = np.stack([-sqz[s] / 2, np.full(ROWS, -0.5, np.float32)])
        in_maps.append({
            "xcat": np.ascontiguousarray(np.concatenate([xt, xt[:, s]], 1)),
            "zcat": np.ascontiguousarray(np.concatenate([zt, zt[:, s]], 1)),
            "acx": np.ascontiguousarray(np.concatenate([axr, axl], 1)),
            "acz": np.ascontiguousarray(np.concatenate([azr, azl], 1)),
            "ident": ident,
        })
    return in_maps


def finish(routs: list[np.ndarray]):
    ss = 0.0
    for c in range(NCORES):
        R = np.asarray(routs[c], np.float64)
        D = R[ROWS:2 * ROWS] - R[:ROWS]
        ss += (D * D).sum()
    loss = np.float32(ss / (N * N))
    return (loss, loss, np.float32(0.0))


def kernel(x: np.ndarray, z: np.ndarray):
    nc = _get_nc()
    in_maps = make_in_maps(x, z)
    res = run_bass_kernel_spmd(nc, in_maps, list(range(NCORES)))
    _CACHE["last_result"] = res
    return finish([res.results[c]["rout"] for c in range(NCORES)])



# revision 7
# speedup vs baseline: 5.2066x; 5.2066x over previous
"""Trainium2 Bass kernel for DPLossV2 soft-rank MSE loss (Fourier method).

Computes, for x:[512,512], z:[512,64]:
    dist_x = cdist(x), dist_z = cdist(z)           (pairwise Euclidean)
    rank_m[i,j] = 1 + sum_k sigmoid(m[i,k]-m[i,j])
    loss = mean((rank_z - rank_x)**2)
returns (loss, loss, 0.0).

Method: the O(n^3) pairwise-difference sigmoid sum is factorized with a
truncated Fourier expansion.  Off-diagonal distances occupy a narrow band
[mu-h, mu+h]; on it
    sigmoid(t) - 1/2 ~= sum_{m=1..M} c_m sin(m*w*t),   w = pi/T,
and sin(m*w*(u_k-u_j)) separates by angle addition, so
    rank[i,j] = const + sum_m [ -c_m(B_m(i)-CB_m) sin(m th_ij)
                                +c_m(A_m(i)-SA_m) cos(m th_ij) ],
    th_ij = w*(S[i,j]-mu),  A_m(i) = sum_k sin(m th_ik), B_m = sum cos,
with SA/CB exact corrections for the clamped diagonal (S[i,i]=0 is
clamped in d^2-space to theta = -THC, a host-known constant).  This
replaces 511 per-k sigmoid instructions with 2M=16 harmonic feature
tiles.  The additive constants cancel in rank_z - rank_x.

Sharding: rows split across 8 cores (64 rows each); per core the x rows
sit on SBUF partitions 0-63 and the z rows on 64-127, so every feature
instruction covers both matrices.  Engines: PE does the Gram/distance
matmuls (f32r, 1 cyc/row) and recombines weighted features straight into
D = rank_z - rank_x via two-band lhsT matmuls accumulating in PSUM;
ScalarE emits harmonics m<=4 directly (Sin table valid to |t|~3.7) plus
the sqrt/theta; DVE and Pool run the Chebyshev step-2 recursions for
m=5..8 in bf16 (s-chain on DVE, c-chain on Pool) and build the weighted
lhsT bands.  Row sums ride free on ACT/tensor_scalar accum_out where
possible, else tensor_reduce.  The masked sum of D^2 is reduced on-chip
to a [64,1] column per core; the host adds 8*64 scalars in f64.

Fit (T, mu, c_m) is data-adaptive, computed on host per call from the
actual distance band; relative loss error ~2e-5 (gate 2e-2).
"""

import numpy as np
from contextlib import ExitStack

import ml_dtypes
import concourse.bass as bass
import concourse.bacc as bacc
import concourse.mybir as mybir
import concourse.tile as tile
from concourse.bass_utils import run_bass_kernel_spmd

N = 512        # number of rows / rank dimension
DX = 512       # x feature dim
DZ = 64        # z feature dim
NCORES = 8
ROWS = N // NCORES          # 64 rows per core
M = 8          # Fourier harmonics
F32 = mybir.dt.float32
F32R = mybir.dt.float32r
BF16 = mybir.dt.bfloat16
AF = mybir.ActivationFunctionType
ALU = mybir.AluOpType
AX = mybir.AxisListType

THB = 0.89     # max |theta| over the off-diag band (design target)
THC = 0.92     # clamp angle for the diagonal entries
GK = DX + DZ + 4   # 580 contraction rows (x feat, z feat, 4 aux)
GW = N + 128       # 640: [rhs columns | 128 block-diag lhsT columns]

# consts layout (per-partition columns)
C_THB, C_DM2, C_OM = 0, 1, 2
C_KSW = 3              # [CB(M) | SA(M)]  (sin side | cos side)
C_CPK = 3 + 2 * M      # [-c(M) | +c(M)]
NCONST = 3 + 4 * M


def _build() -> bass.Bass:
    nc = bacc.Bacc()

    # stacked contraction input: rows 0-511 x features, 512-575 z features,
    # 576-579 aux (ones/sq rows); cols 0-511 rhs, 512-639 block-diag lhsT
    gcat = nc.dram_tensor("gcat", [GK, GW], F32R, kind="ExternalInput")
    cst_d = nc.dram_tensor("cst", [128, NCONST], F32, kind="ExternalInput")
    jb_d = nc.dram_tensor("jband", [128, ROWS], BF16, kind="ExternalInput")
    msk_d = nc.dram_tensor("mask", [ROWS, N], F32, kind="ExternalInput")
    ss_d = nc.dram_tensor("sscol", [ROWS, 1], F32, kind="ExternalOutput")
    import os as _os
    DBG = bool(_os.environ.get("KDBG"))
    if DBG:
        dbg_s = nc.dram_tensor("dbg_s", [128, N], F32, kind="ExternalOutput")
        dbg_ab = nc.dram_tensor("dbg_ab", [128, 2 * M], F32, kind="ExternalOutput")
        dbg_ww = nc.dram_tensor("dbg_ww", [128, 2 * M], F32, kind="ExternalOutput")
        dbg_d = nc.dram_tensor("dbg_d", [ROWS, N], F32, kind="ExternalOutput")
        dbg_s5 = nc.dram_tensor("dbg_s5", [128, N], F32, kind="ExternalOutput")


    with tile.TileContext(nc) as tc:
        with ExitStack() as ctx:
            cp = ctx.enter_context(tc.tile_pool(name="const", bufs=1))
            pp = ctx.enter_context(tc.tile_pool(name="ps", bufs=1, space="PSUM"))

            gb = [cp.tile([min(128, GK - 128 * b), GW], F32R, name=f"gb{b}",
                          tag=f"gb{b}") for b in range(5)]
            cst = cp.tile([128, NCONST], F32, tag="cst")
            jb = cp.tile([128, ROWS], BF16, tag="jb")
            msk = cp.tile([ROWS, N], F32, tag="msk")

            for b in range(4):
                nc.sync.dma_start(gb[b][0:64, :], gcat[b * 128:b * 128 + 64, :])
                nc.sync.dma_start(gb[b][64:128, :], gcat[b * 128 + 64:(b + 1) * 128, :])
            nc.sync.dma_start(gb[4][:], gcat[512:GK, :])
            nc.sync.dma_start(cst[:], cst_d[:])
            nc.sync.dma_start(jb[:], jb_d[:])
            nc.sync.dma_start(msk[:], msk_d[:])

            g_s = pp.tile([128, N], F32, tag="g_s")
            th = pp.tile([128, N], F32, tag="th")
            d_ps = pp.tile([ROWS, N], F32, tag="d_ps")

            # warm the sqrt table set during the DMAs/matmuls
            w0 = cp.tile([1, 1], F32, tag="w0")
            nc.vector.memset(w0[:], 1.0)
            wq = cp.tile([1, 1], F32, tag="wq")
            nc.scalar.activation(wq[:], w0[:], AF.Sqrt)

            # G' = v_i.v_j - sq_i/2 - sq_j/2 for both x and z slabs at once
            # (block-diagonal lhsT; f32r: 1 cyc/row)
            for b in range(5):
                nc.tensor.matmul(g_s[:, :], gb[b][:, N:N + 128], gb[b][:, 0:N],
                                 start=(b == 0), stop=(b == 4))

            # d^2 = max(-2 G', dmin2_p): clamps the diagonal to theta=-THC
            rr = cp.tile([128, N], F32, tag="rr")
            nc.vector.tensor_scalar(rr[:], g_s[:], -2.0, cst[:, C_DM2:C_DM2 + 1],
                                    ALU.mult, ALU.max)
            s_sb = cp.tile([128, N], F32, tag="s_sb")
            nc.scalar.activation(s_sb[:], rr[:], AF.Sqrt)
            # sin table set loads here (trig_and_small), off critical data path
            wsn = cp.tile([1, 1], F32, tag="wsn")
            nc.scalar.activation(wsn[:], w0[:], AF.Sin)
            # theta = om*(S - mu_p)  -> PSUM (ACT reads PSUM faster)
            nc.scalar.activation(th[:], s_sb[:], AF.Identity,
                                 bias=cst[:, C_THB:C_THB + 1],
                                 scale=cst[:, C_OM:C_OM + 1])

            hpi = cp.tile([128, 1], F32, tag="hpi")
            nc.vector.memset(hpi[:], float(np.pi / 2))

            # feature tiles (bf16) and A/B row-sum accumulators (f32)
            s_t = [cp.tile([128, N], BF16, name=f"s{m}", tag=f"s{m}") for m in range(1, M + 1)]
            c_t = [cp.tile([128, N], BF16, name=f"c{m}", tag=f"c{m}") for m in range(1, M + 1)]
            AB = cp.tile([128, 2 * M], F32, tag="AB")

            # direct harmonics on ScalarE (args within Sin table range)
            for m in (1, 2, 3, 4):
                nc.scalar.activation(s_t[m - 1][:], th[:], AF.Sin,
                                     scale=float(m),
                                     accum_out=AB[:, m - 1:m])
            for m in (1, 2, 3):
                nc.scalar.activation(c_t[m - 1][:], th[:], AF.Sin,
                                     scale=float(m), bias=hpi[:],
                                     accum_out=AB[:, M + m - 1:M + m])
            # c4 = 1 - 2 s2^2  (tensor_scalar op1 with an immediate
            # scalar2 drops the constant, so apply -2x+1 via ACT instead)
            q2 = cp.tile([128, N], BF16, tag="q2")
            nc.scalar.activation(q2[:], s_t[1][:], AF.Square)
            one1 = cp.tile([128, 1], F32, tag="one1")
            nc.vector.memset(one1[:], 1.0)
            nc.scalar.activation(c_t[3][:], q2[:], AF.Identity,
                                 bias=one1[:], scale=-2.0,
                                 accum_out=AB[:, M + 3:M + 4])
            # e2 = 2 c2 (recursion multiplier)
            e2 = cp.tile([128, N], BF16, tag="e2")
            nc.gpsimd.tensor_scalar(e2[:], c_t[1][:], 2.0, None, ALU.mult)

            # Chebyshev step-2 recursions: X_m = e2*X_{m-2} - X_{m-4}
            tmps = cp.tile([128, N], BF16, tag="tmps")
            tmpc = cp.tile([128, N], BF16, tag="tmpc")
            for m in range(5, M + 1):
                i = m - 1
                nc.vector.tensor_tensor(tmps[:], e2[:], s_t[i - 2][:], ALU.mult)
                nc.vector.tensor_tensor(s_t[i][:], tmps[:], s_t[i - 4][:],
                                        ALU.subtract)
                nc.vector.tensor_reduce(AB[:, i:i + 1], s_t[i][:],
                                        axis=AX.X, op=ALU.add)
                nc.gpsimd.tensor_tensor(tmpc[:], e2[:], c_t[i - 2][:], ALU.mult)
                nc.gpsimd.tensor_tensor(c_t[i][:], tmpc[:], c_t[i - 4][:],
                                        ALU.subtract)
                nc.vector.tensor_reduce(AB[:, M + i:M + i + 1], c_t[i][:],
                                        axis=AX.X, op=ALU.add)

            # weights WW = (AB_swapped - KSW) * CPACK, in two readiness groups
            WW = cp.tile([128, 2 * M], F32, tag="WW")
            WT = cp.tile([128, 2 * M], F32, tag="WT")

            def weight_group(lo, hi, eng):
                # sin-feature weights m in [lo,hi): use B sums (AB cols M+..)
                eng.tensor_tensor(WT[:, lo:hi], AB[:, M + lo:M + hi],
                                  cst[:, C_KSW + lo:C_KSW + hi], ALU.subtract)
                eng.tensor_tensor(WW[:, lo:hi], WT[:, lo:hi],
                                  cst[:, C_CPK + lo:C_CPK + hi], ALU.mult)
                # cos-feature weights: use A sums
                eng.tensor_tensor(WT[:, M + lo:M + hi], AB[:, lo:hi],
                                  cst[:, C_KSW + M + lo:C_KSW + M + hi],
                                  ALU.subtract)
                eng.tensor_tensor(WW[:, M + lo:M + hi], WT[:, M + lo:M + hi],
                                  cst[:, C_CPK + M + lo:C_CPK + M + hi],
                                  ALU.mult)

            weight_group(0, 4, nc.vector)
            weight_group(4, M, nc.gpsimd)

            # weighted two-band lhsT tiles and the D accumulation on PE
            jw_s = [cp.tile([128, ROWS], BF16, name=f"jws{m}", tag=f"jws{m}") for m in range(M)]
            jw_c = [cp.tile([128, ROWS], BF16, name=f"jwc{m}", tag=f"jwc{m}") for m in range(M)]
            order = [(m, f) for m in range(M) for f in ("s", "c")]
            for idx, (m, f) in enumerate(order):
                col = m if f == "s" else M + m
                jw = jw_s[m] if f == "s" else jw_c[m]
                nc.vector.tensor_scalar(jw[:], jb[:], WW[:, col:col + 1],
                                        None, ALU.mult)
                feat = s_t[m] if f == "s" else c_t[m]
                nc.tensor.matmul(d_ps[:], jw[:], feat[:],
                                 start=(idx == 0), stop=(idx == len(order) - 1))

            # mask diagonal, square, row-reduce; host sums the 64 values
            if DBG:
                s5f = cp.tile([128, N], F32, tag="s5f")
                nc.vector.tensor_copy(s5f[:], s_t[4][:])
                nc.gpsimd.dma_start(dbg_s[:], s_sb[:])
                nc.gpsimd.dma_start(dbg_ab[:], AB[:])
                nc.gpsimd.dma_start(dbg_ww[:], WW[:])
                nc.gpsimd.dma_start(dbg_s5[:], s5f[:])
            dm = cp.tile([ROWS, N], F32, tag="dm")
            nc.vector.tensor_tensor(dm[:], d_ps[:], msk[:], ALU.mult)
            scr = cp.tile([ROWS, N], BF16, tag="scr")
            ss = cp.tile([ROWS, 1], F32, tag="ss")
            nc.scalar.activation(scr[:], dm[:], AF.Square, accum_out=ss[:])
            if DBG:
                nc.gpsimd.dma_start(dbg_d[:], dm[:])
            nc.gpsimd.dma_start(ss_d[:], ss[:])

    nc.compile()
    return nc


_CACHE: dict = {}


def _get_nc() -> bass.Bass:
    if "nc" not in _CACHE:
        _CACHE["nc"] = _build()
    return _CACHE["nc"]


def _fit_coeffs(band_w: float, T: float) -> np.ndarray:
    t = np.linspace(-1.02 * band_w, 1.02 * band_w, 4001)
    target = 1.0 / (1.0 + np.exp(-t)) - 0.5
    A = np.sin(np.outer(t, np.arange(1, M + 1) * np.pi / T))
    c, *_ = np.linalg.lstsq(A, target, rcond=None)
    return c.astype(np.float64)


def _band(a: np.ndarray) -> tuple[float, float]:
    """(min, max) of off-diagonal pairwise distances of rows of a."""
    a = a.astype(np.float64)
    sq = (a * a).sum(1)
    d2 = sq[:, None] + sq[None, :] - 2.0 * (a @ a.T)
    np.fill_diagonal(d2, np.inf)
    lo = np.sqrt(max(d2.min(), 0.0))
    np.fill_diagonal(d2, -np.inf)
    hi = np.sqrt(max(d2.max(), 0.0))
    return lo, hi


def make_in_maps(x: np.ndarray, z: np.ndarray) -> list[dict]:
    x = np.ascontiguousarray(np.asarray(x, np.float32))
    z = np.ascontiguousarray(np.asarray(z, np.float32))
    lox, hix = _band(x)
    loz, hiz = _band(z)
    mux, muz = 0.5 * (lox + hix), 0.5 * (loz + hiz)
    half = max(hix - mux, mux - lox, hiz - muz, muz - loz)
    T = float(np.pi * half / THB)
    om = np.pi / T
    c = _fit_coeffs(2.0 * half, T)
    ms = np.arange(1, M + 1)
    SA = np.sin(-ms * THC)
    CB = np.cos(ms * THC)

    cst = np.zeros((128, NCONST), np.float32)
    cst[0:ROWS, C_THB] = -om * mux
    cst[ROWS:128, C_THB] = -om * muz
    cst[0:ROWS, C_DM2] = (mux - THC / om) ** 2
    cst[ROWS:128, C_DM2] = (muz - THC / om) ** 2
    cst[:, C_OM] = om
    cst[:, C_KSW:C_KSW + M] = CB
    cst[:, C_KSW + M:C_KSW + 2 * M] = SA
    cst[:, C_CPK:C_CPK + M] = -c
    cst[:, C_CPK + M:C_CPK + 2 * M] = c

    jband = np.zeros((128, ROWS), np.float32)
    ar = np.arange(ROWS)
    jband[ar, ar] = -1.0
    jband[ROWS + ar, ar] = 1.0
    jband = jband.astype(ml_dtypes.bfloat16)

    sqx = (x * x).sum(1, dtype=np.float32)
    sqz = (z * z).sum(1, dtype=np.float32)
    xt = np.ascontiguousarray(x.T)
    zt = np.ascontiguousarray(z.T)
    in_maps = []
    for cix in range(NCORES):
        s = slice(cix * ROWS, (cix + 1) * ROWS)
        g = np.zeros((GK, GW), np.float32)
        g[0:DX, 0:N] = xt
        g[DX:DX + DZ, 0:N] = zt
        g[DX + DZ, 0:N] = 1.0
        g[DX + DZ + 1, 0:N] = sqx
        g[DX + DZ + 2, 0:N] = 1.0
        g[DX + DZ + 3, 0:N] = sqz
        g[0:DX, N:N + ROWS] = xt[:, s]
        g[DX:DX + DZ, N + ROWS:N + 128] = zt[:, s]
        g[DX + DZ, N:N + ROWS] = -sqx[s] / 2
        g[DX + DZ + 1, N:N + ROWS] = -0.5
        g[DX + DZ + 2, N + ROWS:N + 128] = -sqz[s] / 2
        g[DX + DZ + 3, N + ROWS:N + 128] = -0.5
        mask = np.ones((ROWS, N), np.float32)
        mask[ar, cix * ROWS + ar] = 0.0
        in_maps.append({
            "gcat": g,
            "cst": cst,
            "jband": jband,
            "mask": mask,
        })
    return in_maps


def finish(sscols: list[np.ndarray]):
    ss = 0.0
    for cix in range(NCORES):
        ss += np.asarray(sscols[cix], np.float64).sum()
    loss = np.float32(ss / (N * N))
    return (loss, loss, np.float32(0.0))


def kernel(x: np.ndarray, z: np.ndarray):
    nc = _get_nc()
    in_maps = make_in_maps(x, z)
    res = run_bass_kernel_spmd(nc, in_maps, list(range(NCORES)))
    _CACHE["last_result"] = res
    return finish([res.results[c]["sscol"] for c in range(NCORES)])


# revision 8
# speedup vs baseline: 8.2368x; 1.5820x over previous
"""Trainium2 Bass kernel for DPLossV2 soft-rank MSE loss (Fourier method).

Computes, for x:[512,512], z:[512,64]:
    dist_x = cdist(x), dist_z = cdist(z)
    rank_m[i,j] = 1 + sum_k sigmoid(m[i,k]-m[i,j])
    loss = mean((rank_z - rank_x)**2)
returns (loss, loss, 0.0).

The O(n^3) pairwise-difference sigmoid sum is factorized with a
truncated Fourier expansion: off-diagonal distances occupy a narrow
band, and on it  sigmoid(t)-1/2 ~= sum_{m<=6} c_m sin(m*w*t); the
angle-addition split turns the per-row sum into 13 feature maps with
per-row weights built from the feature row-sums.  The diagonal
(S[i,i]=0) is clamped in d^2-space to the host-known angle -THC so its
feature values are exact constants, folded into the weight offsets.
Additive rank constants cancel in rank_z - rank_x.

Per core (rows i in an n/8 slab; x rows on partitions 0-63, z rows on
64-127):
  PE     : Gram+norm matmuls (bf16 inputs, f32r aux rows) -> d^2, and
           the recombine sum_f w_f(i) F_f[i,j] folded with the
           (rank_z - rank_x) cross-partition subtraction via two-band
           lhsT tiles, accumulated in PSUM (13 matmuls).
  ScalarE: sqrt, 7 direct Sin maps m<=4 (|arg|<=3.7 table range) with
           free accum_out row-sums, final Square+accum of D.
  DVE    : d^2 clamp, theta, 5 depth-1 products (s2^2, s3^2, c2s3,
           c2c3, s3c3) giving the m=5,6 harmonics by identities with
           the x2/affine factors folded into host constants, 5 row-sum
           reduces, weight algebra, 13 weighted band tiles, diag mask.
  out    : [64,1] masked row-sums of D^2; host adds 512 scalars (f64).

Fit (T, mu, c_m) is data-adaptive per call; rel loss err ~5e-3 with
the 2e-2 gate.
"""

import numpy as np
from contextlib import ExitStack

import ml_dtypes
import concourse.bass as bass
import concourse.bacc as bacc
import concourse.mybir as mybir
import concourse.tile as tile
from concourse.bass_utils import run_bass_kernel_spmd

N = 512
DX = 512
DZ = 64
NCORES = 8
ROWS = N // NCORES
M = 6
F32 = mybir.dt.float32
F32R = mybir.dt.float32r
BF16 = mybir.dt.bfloat16
AF = mybir.ActivationFunctionType
ALU = mybir.AluOpType
AX = mybir.AxisListType

THB = 0.89
THC = 0.92
GK = DX + DZ       # 576 bf16 Gram rows
GW = N + 128       # 640: [rhs | block-diag lhsT]
NT = 13            # feature tiles

# cst columns
C_THB, C_DM2, C_OM = 0, 1, 2
C_KSW = 3
C_CPK = 3 + NT
NCONST = 3 + 2 * NT


def _build() -> bass.Bass:
    nc = bacc.Bacc()

    gcat = nc.dram_tensor("gcat", [GK, GW], BF16, kind="ExternalInput")
    aux = nc.dram_tensor("aux", [4, GW], F32R, kind="ExternalInput")
    cst_d = nc.dram_tensor("cst", [128, NCONST], F32, kind="ExternalInput")
    jb_d = nc.dram_tensor("jband", [128, ROWS], BF16, kind="ExternalInput")
    msk_d = nc.dram_tensor("mask", [ROWS, N], F32, kind="ExternalInput")
    ss_d = nc.dram_tensor("sscol", [ROWS, 1], F32, kind="ExternalOutput")

    with tile.TileContext(nc) as tc:
        with ExitStack() as ctx:
            cp = ctx.enter_context(tc.tile_pool(name="const", bufs=1))
            pp = ctx.enter_context(tc.tile_pool(name="ps", bufs=1, space="PSUM"))

            gb = [cp.tile([128, GW], BF16, name=f"gb{b}", tag=f"gb{b}")
                  for b in range(4)]
            gz = cp.tile([DZ, GW], BF16, tag="gz")
            ab4 = cp.tile([4, GW], F32R, tag="ab4")
            cst = cp.tile([128, NCONST], F32, tag="cst")
            jb = cp.tile([128, ROWS], BF16, tag="jb")
            msk = cp.tile([ROWS, N], F32, tag="msk")

            for b in range(4):
                nc.sync.dma_start(gb[b][0:64, :], gcat[b * 128:b * 128 + 64, :])
                nc.sync.dma_start(gb[b][64:128, :],
                                  gcat[b * 128 + 64:(b + 1) * 128, :])
            nc.sync.dma_start(gz[:], gcat[512:GK, :])
            nc.sync.dma_start(ab4[:], aux[:])
            nc.sync.dma_start(cst[:], cst_d[:])
            nc.sync.dma_start(jb[:], jb_d[:])
            nc.sync.dma_start(msk[:], msk_d[:])

            g_s = pp.tile([128, N], F32, tag="g_s")
            d_ps = pp.tile([ROWS, N], F32, tag="d_ps")

            # warm the sqrt table during DMAs/G'
            w0 = cp.tile([1, 1], F32, tag="w0")
            nc.vector.memset(w0[:], 1.0)
            wq = cp.tile([1, 1], F32, tag="wq")
            nc.scalar.activation(wq[:], w0[:], AF.Sqrt)

            # G' = v_i.v_j - sq_i/2 - sq_j/2 (block-diag lhsT, bf16 + f32r aux)
            for b in range(4):
                nc.tensor.matmul(g_s[:, :], gb[b][:, N:GW], gb[b][:, 0:N],
                                 start=(b == 0), stop=False)
            nc.tensor.matmul(g_s[:, :], gz[:, N:GW], gz[:, 0:N],
                             start=False, stop=False)
            nc.tensor.matmul(g_s[:, :], ab4[:, N:GW], ab4[:, 0:N],
                             start=False, stop=True)

            # d^2 = max(-2 G', dmin2_p); sqrt; theta = om*(S-mu_p)
            rr = cp.tile([128, N], F32, tag="rr")
            nc.vector.tensor_scalar(rr[:], g_s[:], -2.0,
                                    cst[:, C_DM2:C_DM2 + 1],
                                    ALU.mult, ALU.max)
            s_sb = cp.tile([128, N], F32, tag="s_sb")
            nc.scalar.activation(s_sb[:], rr[:], AF.Sqrt)
            wsn = cp.tile([1, 1], F32, tag="wsn")
            nc.scalar.activation(wsn[:], w0[:], AF.Sin)  # trig table load
            th = cp.tile([128, N], F32, tag="th")
            nc.vector.tensor_scalar(th[:], s_sb[:], cst[:, C_OM:C_OM + 1],
                                    cst[:, C_THB:C_THB + 1],
                                    ALU.mult, ALU.add)

            hpi = cp.tile([128, 1], F32, tag="hpi")
            nc.vector.memset(hpi[:], float(np.pi / 2))

            # RAWS: per-tile raw row-sum columns (see host for layout)
            RW = cp.tile([128, NT], F32, tag="RW")
            sc = cp.tile([128, 4], F32, tag="sc")

            s1 = cp.tile([128, N], BF16, tag="s1")
            s2 = cp.tile([128, N], BF16, tag="s2")
            s3 = cp.tile([128, N], BF16, tag="s3")
            s4 = cp.tile([128, N], BF16, tag="s4")
            c1 = cp.tile([128, N], BF16, tag="c1")
            c2 = cp.tile([128, N], BF16, tag="c2")
            c3 = cp.tile([128, N], BF16, tag="c3")
            q2 = cp.tile([128, N], BF16, tag="q2")
            q3 = cp.tile([128, N], BF16, tag="q3")
            p1 = cp.tile([128, N], BF16, tag="p1")
            p2 = cp.tile([128, N], BF16, tag="p2")
            p3 = cp.tile([128, N], BF16, tag="p3")
            onesf = cp.tile([128, N], BF16, tag="onesf")
            nc.vector.memset(onesf[:], -0.5)

            # direct harmonics; order unblocks the products earliest
            nc.scalar.activation(s2[:], th[:], AF.Sin, scale=2.0,
                                 accum_out=RW[:, 5:6])        # A2
            nc.scalar.activation(s3[:], th[:], AF.Sin, scale=3.0,
                                 accum_out=RW[:, 6:7])        # A3
            nc.scalar.activation(c2[:], th[:], AF.Sin, scale=2.0, bias=hpi[:],
                                 accum_out=RW[:, 1:2])        # B2
            nc.scalar.activation(c3[:], th[:], AF.Sin, scale=3.0, bias=hpi[:],
                                 accum_out=RW[:, 2:3])        # B3
            nc.scalar.activation(s1[:], th[:], AF.Sin,
                                 accum_out=RW[:, 4:5])        # A1
            nc.scalar.activation(s4[:], th[:], AF.Sin, scale=4.0,
                                 accum_out=RW[:, 7:8])        # A4
            nc.scalar.activation(c1[:], th[:], AF.Sin, bias=hpi[:],
                                 accum_out=RW[:, 0:1])        # B1

            # depth-1 products (DVE) + row-sum reduces
            nc.vector.tensor_tensor(q2[:], s2[:], s2[:], ALU.mult)
            nc.vector.tensor_reduce(RW[:, 3:4], q2[:], axis=AX.X, op=ALU.add)
            nc.vector.tensor_tensor(q3[:], s3[:], s3[:], ALU.mult)
            nc.vector.tensor_reduce(RW[:, 11:12], q3[:], axis=AX.X, op=ALU.add)
            nc.vector.tensor_tensor(p3[:], s3[:], c3[:], ALU.mult)
            nc.vector.tensor_reduce(RW[:, 8:9], p3[:], axis=AX.X, op=ALU.add)
            nc.vector.tensor_tensor(p1[:], c2[:], s3[:], ALU.mult)
            nc.vector.tensor_reduce(sc[:, 0:1], p1[:], axis=AX.X, op=ALU.add)
            nc.vector.tensor_tensor(p2[:], c2[:], c3[:], ALU.mult)
            nc.vector.tensor_reduce(sc[:, 1:2], p2[:], axis=AX.X, op=ALU.add)
            # Bx = 2*sum(p2) - B1 ; Ax = 2*sum(p1) - A1
            nc.vector.tensor_scalar(sc[:, 2:3], sc[:, 1:2], 2.0, None, ALU.mult)
            nc.vector.tensor_tensor(RW[:, 9:10], sc[:, 2:3], RW[:, 0:1],
                                    ALU.subtract)
            nc.vector.tensor_scalar(sc[:, 3:4], sc[:, 0:1], 2.0, None, ALU.mult)
            nc.vector.tensor_tensor(RW[:, 10:11], sc[:, 3:4], RW[:, 4:5],
                                    ALU.subtract)

            # weights: WW = (RW - KSW) * CPK, in readiness groups
            WT = cp.tile([128, NT], F32, tag="WT")
            WW = cp.tile([128, NT], F32, tag="WW")

            def wgrp(lo, hi):
                nc.vector.tensor_tensor(WT[:, lo:hi], RW[:, lo:hi],
                                        cst[:, C_KSW + lo:C_KSW + hi],
                                        ALU.subtract)
                nc.vector.tensor_tensor(WW[:, lo:hi], WT[:, lo:hi],
                                        cst[:, C_CPK + lo:C_CPK + hi],
                                        ALU.mult)

            wgrp(1, 3)    # s2, s3
            wgrp(5, 8)    # c2, c3, q2
            wgrp(0, 1)    # s1 pre-combo (ws1)
            wgrp(3, 5)    # s4, c1 pre-combo (wc1)
            wgrp(8, 12)   # q3, P1, P2, P3
            # post combos: s1 -= WW[P1]/2 ; c1 -= WW[P2]/2 ; ones = q2c+q3c
            nc.vector.tensor_scalar(WT[:, 9:10], WW[:, 9:10], 0.5, None,
                                    ALU.mult)
            nc.vector.tensor_tensor(WT[:, 0:1], WW[:, 0:1], WT[:, 9:10],
                                    ALU.subtract)
            nc.vector.tensor_scalar(WT[:, 10:11], WW[:, 10:11], 0.5, None,
                                    ALU.mult)
            nc.vector.tensor_tensor(WT[:, 4:5], WW[:, 4:5], WT[:, 10:11],
                                    ALU.subtract)
            nc.vector.tensor_tensor(WT[:, 12:13], WW[:, 7:8], WW[:, 8:9],
                                    ALU.add)

            # weighted two-band lhsT tiles + D accumulation (PE)
            feats = [s2, s3, c2, c3, q2,
                     s1, s4, c1, q3, p1, p2, p3, onesf]
            wsrc = [(WW, 1), (WW, 2), (WW, 5), (WW, 6), (WW, 7),
                    (WT, 0), (WW, 3), (WT, 4), (WW, 8), (WW, 9),
                    (WW, 10), (WW, 11), (WT, 12)]
            jw = [cp.tile([128, ROWS], BF16, name=f"jw{i}", tag=f"jw{i}")
                  for i in range(NT)]
            for i, (ft, (wt, col)) in enumerate(zip(feats, wsrc)):
                nc.vector.tensor_scalar(jw[i][:], jb[:], wt[:, col:col + 1],
                                        None, ALU.mult)
                nc.tensor.matmul(d_ps[:], jw[i][:], ft[:],
                                 start=(i == 0), stop=(i == NT - 1))

            # mask diagonal, square, row-reduce
            dm = cp.tile([ROWS, N], F32, tag="dm")
            nc.vector.tensor_tensor(dm[:], d_ps[:], msk[:], ALU.mult)
            scr = cp.tile([ROWS, N], BF16, tag="scr")
            ss = cp.tile([ROWS, 1], F32, tag="ss")
            nc.scalar.activation(scr[:], dm[:], AF.Square, accum_out=ss[:])
            nc.sync.dma_start(ss_d[:], ss[:])

    nc.compile()
    return nc


_CACHE: dict = {}


def _get_nc() -> bass.Bass:
    if "nc" not in _CACHE:
        _CACHE["nc"] = _build()
    return _CACHE["nc"]


def _fit_coeffs(band_w: float, T: float) -> np.ndarray:
    t = np.linspace(-1.02 * band_w, 1.02 * band_w, 4001)
    target = 1.0 / (1.0 + np.exp(-t)) - 0.5
    A = np.sin(np.outer(t, np.arange(1, M + 1) * np.pi / T))
    c, *_ = np.linalg.lstsq(A, target, rcond=None)
    return c.astype(np.float64)


def _band(a: np.ndarray) -> tuple[float, float]:
    a = a.astype(np.float64)
    sq = (a * a).sum(1)
    d2 = sq[:, None] + sq[None, :] - 2.0 * (a @ a.T)
    np.fill_diagonal(d2, np.inf)
    lo = np.sqrt(max(d2.min(), 0.0))
    np.fill_diagonal(d2, -np.inf)
    hi = np.sqrt(max(d2.max(), 0.0))
    return lo, hi


def make_in_maps(x: np.ndarray, z: np.ndarray) -> list[dict]:
    x = np.ascontiguousarray(np.asarray(x, np.float32))
    z = np.ascontiguousarray(np.asarray(z, np.float32))
    lox, hix = _band(x)
    loz, hiz = _band(z)
    mux, muz = 0.5 * (lox + hix), 0.5 * (loz + hiz)
    half = max(hix - mux, mux - lox, hiz - muz, muz - loz)
    half *= 1.01   # bf16 Gram inputs perturb distances slightly
    T = float(np.pi * half / THB)
    om = np.pi / T
    c = _fit_coeffs(2.0 * half, T)
    ms = np.arange(1, M + 1)
    SA = np.sin(-ms * THC)
    CB = np.cos(ms * THC)
    c1_, c2_, c3_, c4_, c5_, c6_ = c

    cst = np.zeros((128, NCONST), np.float32)
    cst[0:ROWS, C_THB] = -om * mux
    cst[ROWS:128, C_THB] = -om * muz
    cst[0:ROWS, C_DM2] = (mux - THC / om) ** 2
    cst[ROWS:128, C_DM2] = (muz - THC / om) ** 2
    cst[:, C_OM] = om

    # tile order: s1 s2 s3 s4 c1 c2 c3 q2 q3 P1 P2 P3 ones
    KSW = [CB[0], CB[1], CB[2], (N - CB[3]) / 2,
           SA[0], SA[1], SA[2], SA[3],
           SA[5] / 2, CB[4], SA[4], (N - CB[5]) / 2, 0.0]
    CPK = [-c1_, -c2_, -c3_, 2 * c4_,
           c1_, c2_, c3_, -2 * c4_,
           -4 * c6_, -2 * c5_, 2 * c5_, 4 * c6_, 0.0]
    cst[:, C_KSW:C_KSW + NT] = np.array(KSW, np.float32)
    cst[:, C_CPK:C_CPK + NT] = np.array(CPK, np.float32)

    jband = np.zeros((128, ROWS), np.float32)
    ar = np.arange(ROWS)
    jband[ar, ar] = -1.0
    jband[ROWS + ar, ar] = 1.0
    jband = jband.astype(ml_dtypes.bfloat16)

    sqx = ((x.astype(np.float64)) ** 2).sum(1).astype(np.float32)
    sqz = ((z.astype(np.float64)) ** 2).sum(1).astype(np.float32)
    xt = np.ascontiguousarray(x.T)
    zt = np.ascontiguousarray(z.T)
    in_maps = []
    for cix in range(NCORES):
        s = slice(cix * ROWS, (cix + 1) * ROWS)
        g = np.zeros((GK, GW), np.float32)
        g[0:DX, 0:N] = xt
        g[DX:GK, 0:N] = zt
        g[0:DX, N:N + ROWS] = xt[:, s]
        g[DX:GK, N + ROWS:N + 128] = zt[:, s]
        gb16 = np.ascontiguousarray(g).astype(ml_dtypes.bfloat16)
        a4 = np.zeros((4, GW), np.float32)
        a4[0, 0:N] = 1.0
        a4[1, 0:N] = sqx
        a4[2, 0:N] = 1.0
        a4[3, 0:N] = sqz
        a4[0, N:N + ROWS] = -sqx[s] / 2
        a4[1, N:N + ROWS] = -0.5
        a4[2, N + ROWS:N + 128] = -sqz[s] / 2
        a4[3, N + ROWS:N + 128] = -0.5
        mask = np.ones((ROWS, N), np.float32)
        mask[ar, cix * ROWS + ar] = 0.0
        in_maps.append({
            "gcat": gb16,
            "aux": a4,
            "cst": cst,
            "jband": jband,
            "mask": mask,
        })
    return in_maps


def finish(sscols: list[np.ndarray]):
    ss = 0.0
    for cix in range(NCORES):
        ss += np.asarray(sscols[cix], np.float64).sum()
    loss = np.float32(ss / (N * N))
    return (loss, loss, np.float32(0.0))


def kernel(x: np.ndarray, z: np.ndarray):
    nc = _get_nc()
    in_maps = make_in_maps(x, z)
    res = run_bass_kernel_spmd(nc, in_maps, list(range(NCORES)))
    _CACHE["last_result"] = res
    return finish([res.results[c]["sscol"] for c in range(NCORES)])
